# revision 19
# baseline (speedup 1.0000x reference)
"""Trainium2 Bass kernel for nn_CacheAttention (16-head causal MHA, T=2048 B=4 E=1024).

Sharding: 16 heads split across 8 NeuronCores (2 heads / core).  Each core
projects q/k/v with its 128-column slice of the weights, runs attention for
its 8 (batch, head) pairs, applies its 128-row slice of wo, and stores a
partial [B*T, E] output; the host sums the 8 partials and adds the output
bias (with bv @ wo.T folded in on the host, since softmax rows sum to 1).

Structure (per core, software-pipelined across batches):
  - q/k projected into head-transposed layout [d, T] (d on partitions);
    v projected directly into natural layout [s, d] per 128-token s-tile
    with a ones column appended per head (softmax denominator for free).
  - scores computed transposed (scores^T = K_tile.T @ Q) per s-tile with
    both heads packed in one [128, 1024] PSUM tile; exp on ACT covers both
    heads in one instruction.  The causal mask is applied only on diagonal
    128x128 blocks, as an additive -1e9 triangle accumulated into PSUM by an
    identity-lhsT matmul; off-diagonal masked tiles are skipped entirely by
    per-q-subtile PV bounds.
  - PV in natural layout: out[q, d] accumulated over s-tiles j <= Q with the
    denominator in column 64; normalization is a per-partition tensor_scalar
    (PSUM->SBUF move), then a PE transpose puts attention back in [d, q]
    for the output projection.
  - projections of batch b+1 and the output projection of the previous chunk
    are emitted interleaved into batch b's attention s-tile loop, so the PE's
    in-order queue always has work while ACT runs exp.
"""

import sys

if "/opt/trn_rl_repo" not in sys.path:
    sys.path.insert(0, "/opt/trn_rl_repo")

import numpy as np
import ml_dtypes

import concourse.mybir as mybir
import concourse.tile as tile
from concourse import bacc
from concourse.bass_utils import run_bass_kernel_spmd
from concourse.masks import make_identity

BF16 = ml_dtypes.bfloat16
F32 = mybir.dt.float32
BF = mybir.dt.bfloat16

T, B, E = 2048, 4, 1024
H, D = 16, 64
NCORES = 8
HPC = H // NCORES          # heads per core = 2
DC = HPC * D               # head-dim columns per core = 128
R = B * T                  # rows (b-major: r = b*T + t) = 8192
KT = E // 128              # E contraction tiles = 8
NCH = T // 512             # q chunks per (b,h) pair = 4
NST = T // 128             # s tiles per (b,h) pair = 16
SCALE = float(D) ** -0.5
NEG = -1.0e9

_CACHE = {}


def _build(causal: bool, reps: int = 1, variant: str = "base"):
    nc = bacc.Bacc("TRN2", target_bir_lowering=False, debug=False, num_devices=NCORES)

    qT_d = nc.dram_tensor("qT", [E, R], BF, kind="ExternalInput")
    kT_d = nc.dram_tensor("kT", [E, R], BF, kind="ExternalInput")
    vT_d = nc.dram_tensor("vT", [E, R], BF, kind="ExternalInput")
    wqT_d = nc.dram_tensor("wqT", [E, DC], BF, kind="ExternalInput")
    wkT_d = nc.dram_tensor("wkT", [E, DC], BF, kind="ExternalInput")
    wvT_d = nc.dram_tensor("wvT", [E, DC], BF, kind="ExternalInput")
    woT_d = nc.dram_tensor("woT", [DC, E], BF, kind="ExternalInput")
    bq_d = nc.dram_tensor("bq", [DC, 1], F32, kind="ExternalInput")
    bk_d = nc.dram_tensor("bk", [DC, 1], F32, kind="ExternalInput")
    if causal:
        tri_d = nc.dram_tensor("tri", [128, 128], BF, kind="ExternalInput")
    else:
        em_d = nc.dram_tensor("emaskT", [T, T], BF, kind="ExternalInput")
    out_d = nc.dram_tensor("out", [R, E], BF, kind="ExternalOutput")

    Exp = mybir.ActivationFunctionType.Exp
    add = mybir.AluOpType.add
    mult = mybir.AluOpType.mult

    NB = B * reps
    src_map = {"q": qT_d, "k": kT_d, "v": vT_d}

    with tile.TileContext(nc) as tc:
        with (
            tc.tile_pool(name="wp", bufs=1) as wp,
            tc.tile_pool(name="mp", bufs=2) as mp,
            tc.tile_pool(name="ps", bufs=2, space="PSUM") as ps,
        ):
            # ---- constants / weights (persistent) ----
            wq_sb = wp.tile([128, KT, DC], BF, tag="wq")
            wk_sb = wp.tile([128, KT, DC], BF, tag="wk")
            wv_sb = wp.tile([128, KT, DC], BF, tag="wv")
            bq_sb = wp.tile([DC, 1], F32, tag="bq")
            bk_sb = wp.tile([DC, 1], F32, tag="bk")
            wo_sb = wp.tile([DC, E], BF, tag="wo")
            # wq + bq first so the first projection piece can start ASAP;
            # xin(0, 0) is issued right after wq (see prologue below)
            nc.sync.dma_start(wq_sb, wqT_d.ap().rearrange("(k p) d -> p k d", p=128))
            nc.sync.dma_start(bq_sb, bq_d.ap())
            preamble_rest = []
            preamble_rest.append(lambda: nc.sync.dma_start(
                wk_sb, wkT_d.ap().rearrange("(k p) d -> p k d", p=128)))
            preamble_rest.append(lambda: nc.sync.dma_start(bk_sb, bk_d.ap()))
            preamble_rest.append(lambda: nc.sync.dma_start(
                wv_sb, wvT_d.ap().rearrange("(k p) d -> p k d", p=128)))
            preamble_rest.append(lambda: nc.sync.dma_start(wo_sb, woT_d.ap()))
            ident = wp.tile([128, 128], BF, tag="ident")
            make_identity(nc, ident)
            ident32 = wp.tile([128, 128], F32, tag="ident32")
            make_identity(nc, ident32)
            if causal:
                tri_sb = wp.tile([128, 128], BF, tag="tri")
                preamble_rest.append(lambda: nc.sync.dma_start(tri_sb, tri_d.ap()))

            # ---- per-batch persistent-ish tiles (rotated via pool bufs) ----
            qT = {}    # bb -> [128 d, T] bf16
            kT = {}
            vn = {}    # bb -> [128 s-part, NST*130] bf16  (v | 1 | v | 1 per s-tile)
            atT = {}   # bb -> [128 d, T] bf16 attention output, transposed
            xin = {}   # (bb, c, t) -> [128, KT, 512] bf16

            def issue_xin(bb, c):
                if bb >= NB or (bb, c, "q") in xin:
                    return
                b = bb % B
                for t in ("q", "k", "v"):
                    xt = mp.tile([128, KT, 512], BF, tag=f"x{t}", bufs=3,
                                 name=f"x{t}_{bb}_{c}")
                    src = src_map[t].ap().rearrange("(k p) r -> p k r", p=128)
                    nc.sync.dma_start(
                        xt, src[:, :, b * T + 512 * c : b * T + 512 * (c + 1)]
                    )
                    xin[(bb, c, t)] = xt

            def proj_qk_piece(bb, c, t):
                # q/k projection for column chunk c -> qT/kT[bb][:, 512c:512c+512]
                def emit():
                    w_sb = wq_sb if t == "q" else wk_sb
                    bias = bq_sb if t == "q" else bk_sb
                    scale = SCALE if t == "q" else 1.0
                    dst = qT[bb] if t == "q" else kT[bb]
                    xt = xin.pop((bb, c, t))
                    pps = ps.tile([128, 512], F32, tag="pp", bufs=1, name=f"pp_{t}{bb}{c}")
                    for k in range(KT):
                        nc.tensor.matmul(
                            pps, w_sb[:, k, :], xt[:, k, :],
                            start=(k == 0), stop=(k == KT - 1),
                        )
                    nc.vector.tensor_scalar(
                        dst[:, 512 * c : 512 * (c + 1)], pps, bias, scale, add, mult
                    )
                return emit

            def proj_v_piece(bb, c, jjs):
                # v projection for s-tiles [4c+jj for jj in jjs] -> vn[bb] natural
                def emit():
                    xt = xin[(bb, c, "v")]
                    pps = ps.tile([128, 512], F32, tag="pp", bufs=1, name=f"pp_v{bb}{c}{jjs[0]}")
                    for jj in jjs:
                        for k in range(KT):
                            nc.tensor.matmul(
                                pps[:, 128 * jj : 128 * (jj + 1)],
                                xt[:, k, 128 * jj : 128 * (jj + 1)],
                                wv_sb[:, k, :],
                                start=(k == 0), stop=(k == KT - 1),
                            )
                    for jj in jjs:
                        j = 4 * c + jj
                        # both heads in one strided op (ones col skipped)
                        dstv = vn[bb][:, 130 * j : 130 * j + 130].rearrange(
                            "p (two c2) -> p two c2", two=2)[:, :, 0:64]
                        srcv = pps[:, 128 * jj : 128 * (jj + 1)].rearrange(
                            "p (two c2) -> p two c2", two=2)
                        nc.vector.tensor_copy(dstv, srcv)
                    if jjs[-1] == 3 and c == NCH - 1:
                        pass
                return emit

            def alloc_batch(bb):
                qT[bb] = mp.tile([DC, T], BF, tag="qTb", name=f"qT{bb}")
                kT[bb] = mp.tile([DC, T], BF, tag="kTb", name=f"kT{bb}")
                vn[bb] = mp.tile([128, NST * 130], BF, tag="vnat", name=f"vn{bb}")
                atT[bb] = mp.tile([DC, T], BF, tag="atT", name=f"atT{bb}")
                vv = vn[bb].rearrange("p (j c) -> p j c", c=65)
                nc.vector.memset(vv[:, :, 64], 1.0)

            def proj_pieces(bb):
                # all projection pieces for batch bb, chunk-major
                out = []
                for c in range(NCH):
                    out.append(proj_qk_piece(bb, c, "q"))
                    out.append(proj_qk_piece(bb, c, "k"))
                    out.append(proj_v_piece(bb, c, (0, 1)))
                    out.append(proj_v_piece(bb, c, (2, 3)))
                return out

            osb_cur = [None]

            def outproj_piece(bb, rr, n):
                # half n of the output projection for global r-tile rr
                def emit():
                    b = bb % B
                    ops_t = ps.tile([128, 512], F32, tag="pt" if n == 0 else "pp",
                                    bufs=1, name=f"ops{bb}_{rr}_{n}")
                    nc.tensor.matmul(
                        ops_t,
                        atT[bb][:, 128 * rr : 128 * (rr + 1)],
                        wo_sb[:, 512 * n : 512 * (n + 1)],
                        start=True, stop=True,
                    )
                    if n == 0:
                        osb_cur[0] = mp.tile([128, E], BF, tag="osb", bufs=4,
                                             name=f"osb{bb}_{rr}")
                    o_sb = osb_cur[0]
                    nc.vector.tensor_copy(o_sb[:, 512 * n : 512 * (n + 1)], ops_t)
                    if n == 1:
                        nc.sync.dma_start(
                            out_d.ap()[b * T + 128 * rr : b * T + 128 * (rr + 1), :],
                            o_sb,
                        )
                return emit

            def attn_chunk(bb, c, pieces):
                # attention for q-chunk c of batch bb; `pieces` are deferred
                # emitters (projections of bb+1, outproj of previous chunk)
                # interleaved into the s-tile loop to keep PE busy during exp.
                n_s = 4 * (c + 1) if causal else NST
                pi = 0
                # spread pieces across the s-tile loop
                for j in range(n_s):
                    diag = causal and j >= 4 * c
                    sc = ps.tile([128, 1024], F32, tag="sc", name=f"sc{bb}_{c}_{j}")
                    for h in range(HPC):
                        nc.tensor.matmul(
                            sc[:, 512 * h : 512 * (h + 1)],
                            kT[bb][64 * h : 64 * h + 64, 128 * j : 128 * (j + 1)],
                            qT[bb][64 * h : 64 * h + 64, 512 * c : 512 * (c + 1)],
                            start=True, stop=True,
                        )
                    if diag:
                        qo = 128 * (j - 4 * c)
                        for h in range(HPC):
                            nc.tensor.matmul(
                                sc[:, 512 * h + qo : 512 * h + qo + 128],
                                ident, tri_sb,
                                start=False, stop=True, skip_group_check=True,
                            )
                    pT = mp.tile([128, 1024], BF, tag="pT", bufs=4,
                                 name=f"pT{bb}_{c}_{j}")
                    nc.scalar.activation(pT, sc, Exp)
                    if not causal:
                        em = mp.tile([128, 512], BF, tag="em", bufs=3,
                                     name=f"em{bb}_{c}_{j}")
                        nc.sync.dma_start(
                            em,
                            em_d.ap()[128 * j : 128 * (j + 1),
                                      512 * c : 512 * (c + 1)],
                        )
                        pm = mp.tile([128, 1024], BF, tag="pm", bufs=4,
                                     name=f"pm{bb}_{c}_{j}")
                        for h in range(HPC):
                            nc.vector.tensor_tensor(
                                pm[:, 512 * h : 512 * (h + 1)],
                                pT[:, 512 * h : 512 * (h + 1)], em, mult,
                            )
                        pT = pm
                    # interleave deferred pieces while ACT runs exp; spread
                    # them so none bunch up at the end of short chunks
                    want = -(-(len(pieces) - pi) // (n_s - j)) if j < n_s else 0
                    for _ in range(max(want, 0)):
                        if pi < len(pieces):
                            pieces[pi]()
                            pi += 1
                    # PV: natural layout, per-q-subtile causal bounds.
                    # anat groups share PSUM banks: only the first group of
                    # each bank does start=True (marks the 2KB zero region);
                    # the other groups' first writes zero their own bytes via
                    # the pending-zero mechanism.
                    for h in range(HPC):
                        for qq in range(4):
                            Q = 4 * c + qq
                            if causal and j > Q:
                                continue
                            g = 2 * qq + h
                            last = (j == Q) if causal else (j == n_s - 1)
                            nc.tensor.matmul(
                                anat[:, 128 * g : 128 * g + 65],
                                pT[:, 512 * h + 128 * qq : 512 * h + 128 * (qq + 1)],
                                vn[bb][:, 130 * j + 65 * h : 130 * j + 65 * (h + 1)],
                                start=(j == 0 and g in (0, 4)), stop=last,
                                skip_group_check=True,
                            )
                    # normalization for finished q-subtiles
                    qq_done = []
                    if causal and j >= 4 * c:
                        qq_done = [j - 4 * c]
                    elif not causal and j == n_s - 1:
                        qq_done = [0, 1, 2, 3]
                    for qq in qq_done:
                        a_sb = mp.tile([128, 128], F32, tag="asb", name=f"asb{bb}_{c}_{qq}")
                        rl = mp.tile([128, 2], F32, tag="rl", name=f"rl{bb}_{c}_{qq}")
                        denoms = anat.rearrange("p (g c2) -> p g c2", c2=128)[
                            :, 2 * qq : 2 * qq + 2, 64
                        ]
                        with nc.allow_low_precision(reason="softmax denom recip"):
                            nc.vector.reciprocal(rl, denoms)
                        for h in range(HPC):
                            g = 2 * qq + h
                            nc.vector.tensor_scalar(
                                a_sb[:, 64 * h : 64 * (h + 1)],
                                anat[:, 128 * g : 128 * g + 64],
                                rl[:, h : h + 1], None, mult,
                            )
                        if qq == 0:
                            ptr[0] = ps.tile([128, 512], F32, tag="pt", bufs=1,
                                             name=f"pt{bb}_{c}")
                        nc.tensor.transpose(
                            ptr[0][:, 128 * qq : 128 * (qq + 1)], a_sb, ident32
                        )
                        if qq == 3:
                            nc.vector.tensor_copy(
                                atT[bb][:, 512 * c : 512 * (c + 1)], ptr[0]
                            )
                # leftover pieces
                while pi < len(pieces):
                    pieces[pi]()
                    pi += 1

            # ---------------- main pipelined schedule ----------------
            ptr = [None]
            # prologue: batch 0 loaded and projected up front; remaining
            # weight DMAs interleaved behind the first xin chunk
            issue_xin(0, 0)
            for fn in preamble_rest:
                fn()
            for c in range(1, NCH):
                issue_xin(0, c)
            issue_xin(1, 0)
            alloc_batch(0)
            anat = ps.tile([128, 1024], F32, tag="anat", bufs=1, name="anat")
            for p in proj_pieces(0):
                p()

            pending_outproj = []   # r-tile pieces of the previous chunk
            for bb in range(NB):
                if bb + 1 < NB:
                    alloc_batch(bb + 1)
                    next_proj = proj_pieces(bb + 1)
                else:
                    next_proj = []
                for c in range(NCH):
                    # xin prefetch for the window after next
                    if c < NCH - 1:
                        issue_xin(bb + 1, c + 1)
                    else:
                        issue_xin(bb + 2, 0)
                    # interleave outproj halves with projection pieces of
                    # batch bb+1 so consecutive pieces use different PSUM tags
                    take = list(next_proj[4 * c : 4 * (c + 1)])
                    ops = list(pending_outproj)
                    pieces = []
                    while take or ops:
                        if take:
                            pieces.append(take.pop(0))
                        for _ in range(2):
                            if ops:
                                pieces.append(ops.pop(0))
                    attn_chunk(bb, c, pieces)
                    pending_outproj = [
                        outproj_piece(bb, 4 * c + r, n)
                        for r in range(4) for n in range(2)
                    ]
            for p in pending_outproj:
                p()

    nc.compile()
    return nc


def _causal_mask_ref():
    return np.where(
        np.arange(T)[:, None] >= np.arange(T)[None, :], np.float32(0.0), np.float32(-1e9)
    ).astype(np.float32)


def _tri_pattern():
    # additive causal triangle for a diagonal 128x128 block:
    # tri[s, q] = 0 if s <= q else NEG
    s = np.arange(128)[:, None]
    q = np.arange(128)[None, :]
    return np.where(s <= q, np.float32(0.0), np.float32(NEG)).astype(BF16)


def _prep_in_maps(query, key, value, attn_mask, wq, bq, wk, bk, wv, bv, wo, causal):
    # [T, B, E] -> [E, B*T] b-major columns, bf16
    qT = np.ascontiguousarray(query.transpose(2, 1, 0).reshape(E, R)).astype(BF16)
    kT = np.ascontiguousarray(key.transpose(2, 1, 0).reshape(E, R)).astype(BF16)
    vT = np.ascontiguousarray(value.transpose(2, 1, 0).reshape(E, R)).astype(BF16)
    common = {"qT": qT, "kT": kT, "vT": vT}
    if causal:
        common["tri"] = np.ascontiguousarray(_tri_pattern())
    else:
        common["emaskT"] = np.exp(attn_mask.astype(np.float64).T).astype(BF16)
    in_maps = []
    for c in range(NCORES):
        sl = slice(DC * c, DC * (c + 1))
        m = dict(common)
        m["wqT"] = np.ascontiguousarray(wq[sl, :].T).astype(BF16)
        m["wkT"] = np.ascontiguousarray(wk[sl, :].T).astype(BF16)
        m["wvT"] = np.ascontiguousarray(wv[sl, :].T).astype(BF16)
        m["woT"] = np.ascontiguousarray(wo[:, sl].T).astype(BF16)
        m["bq"] = bq[sl].astype(np.float32)[:, None]
        m["bk"] = bk[sl].astype(np.float32)[:, None]
        in_maps.append(m)
    return in_maps


def _postprocess(results, bo_eff):
    acc = results[0]["out"].astype(np.float32)
    for c in range(1, NCORES):
        acc = acc + results[c]["out"].astype(np.float32)
    out = acc.reshape(B, T, E).transpose(1, 0, 2) + bo_eff[None, None, :]
    return np.ascontiguousarray(out.astype(np.float32))


def kernel(query, key, value, attn_mask, wq, bq, wk, bk, wv, bv, wo, bo):
    assert query.shape == (T, B, E), query.shape
    causal = bool(np.array_equal(attn_mask, _causal_mask_ref()))
    if causal not in _CACHE:
        _CACHE[causal] = _build(causal)
    nc = _CACHE[causal]
    in_maps = _prep_in_maps(
        query, key, value, attn_mask, wq, bq, wk, bk, wv, bv, wo, causal
    )
    res = run_bass_kernel_spmd(nc, in_maps, core_ids=list(range(NCORES)))
    # bv passes through softmax unchanged (rows sum to 1), so its effect on
    # the output is the constant bv @ wo.T — folded into the output bias.
    bo_eff = (
        np.asarray(bo, dtype=np.float64)
        + np.asarray(bv, dtype=np.float64) @ np.asarray(wo, dtype=np.float64).T
    ).astype(np.float32)
    return _postprocess(res.results, bo_eff)


# revision 24
# speedup vs baseline: 1.3205x; 1.3205x over previous
"""Trainium2 Bass kernel for nn_CacheAttention (16-head causal MHA, T=2048 B=4 E=1024).

Sharding: 16 heads split across 8 NeuronCores (2 heads / core).  Each core
projects q/k/v with its 128-column slice of the weights, runs attention for
its 8 (batch, head) pairs, applies its 128-row slice of wo, and stores a
partial [B*T, E] output; the host sums the 8 partials and adds the output
bias (with bv @ wo.T folded in on the host, since softmax rows sum to 1).

Structure (per core, software-pipelined across batches):
  - q/k projected into head-transposed layout [d, T] (d on partitions);
    v projected directly into natural layout [s, d] per 128-token s-tile
    with a ones column appended per head (softmax denominator for free).
  - scores computed transposed (scores^T = K_tile.T @ Q) per s-tile with
    both heads packed in one [128, 1024] PSUM tile; exp on ACT covers both
    heads in one instruction.  The causal mask is applied only on diagonal
    diagonal s-tiles as additive -1e9 patterns accumulated into PSUM by an
    identity-lhsT matmul; fully-masked tiles are skipped by chunk bounds.
  - PV in transposed layout: out^T [65, 512] per head accumulated over all
    s-tiles (masked probabilities are exactly 0), with the softmax denominator
    in row 64 via a ones column in v; normalization is reciprocal ->
    partition_broadcast -> multiply, writing attention straight into [d, q]
    layout for the output projection.
  - projections of batch b+1 and the output projection of the previous chunk
    are emitted interleaved into batch b's attention s-tile loop, so the PE's
    in-order queue always has work while ACT runs exp.
"""

import sys

if "/opt/trn_rl_repo" not in sys.path:
    sys.path.insert(0, "/opt/trn_rl_repo")

import numpy as np
import ml_dtypes

import concourse.mybir as mybir
import concourse.tile as tile
from concourse import bacc
from concourse.bass_utils import run_bass_kernel_spmd
from concourse.masks import make_identity

BF16 = ml_dtypes.bfloat16
F32 = mybir.dt.float32
BF = mybir.dt.bfloat16

T, B, E = 2048, 4, 1024
H, D = 16, 64
NCORES = 8
HPC = H // NCORES          # heads per core = 2
DC = HPC * D               # head-dim columns per core = 128
R = B * T                  # rows (b-major: r = b*T + t) = 8192
KT = E // 128              # E contraction tiles = 8
NCH = T // 512             # q chunks per (b,h) pair = 4
NST = T // 128             # s tiles per (b,h) pair = 16
SCALE = float(D) ** -0.5
NEG = -1.0e9

_CACHE = {}


def _build(causal: bool, reps: int = 1, variant: str = "base"):
    nc = bacc.Bacc("TRN2", target_bir_lowering=False, debug=False, num_devices=NCORES)

    qT_d = nc.dram_tensor("qT", [E, R], BF, kind="ExternalInput")
    kT_d = nc.dram_tensor("kT", [E, R], BF, kind="ExternalInput")
    vT_d = nc.dram_tensor("vT", [E, R], BF, kind="ExternalInput")
    wqT_d = nc.dram_tensor("wqT", [E, DC], BF, kind="ExternalInput")
    wkT_d = nc.dram_tensor("wkT", [E, DC], BF, kind="ExternalInput")
    wvT_d = nc.dram_tensor("wvT", [E, DC], BF, kind="ExternalInput")
    woT_d = nc.dram_tensor("woT", [DC, E], BF, kind="ExternalInput")
    bq_d = nc.dram_tensor("bq", [DC, 1], F32, kind="ExternalInput")
    bk_d = nc.dram_tensor("bk", [DC, 1], F32, kind="ExternalInput")
    if causal:
        tri_d = nc.dram_tensor("tri", [128, 128], BF, kind="ExternalInput")
    else:
        em_d = nc.dram_tensor("emaskT", [T, T], BF, kind="ExternalInput")
    out_d = nc.dram_tensor("out", [R, E], BF, kind="ExternalOutput")

    Exp = mybir.ActivationFunctionType.Exp
    add = mybir.AluOpType.add
    mult = mybir.AluOpType.mult

    NB = B * reps
    src_map = {"q": qT_d, "k": kT_d, "v": vT_d}

    with tile.TileContext(nc) as tc:
        with (
            tc.tile_pool(name="wp", bufs=1) as wp,
            tc.tile_pool(name="mp", bufs=2) as mp,
            tc.tile_pool(name="ps", bufs=2, space="PSUM") as ps,
        ):
            # ---- constants / weights (persistent) ----
            wq_sb = wp.tile([128, KT, DC], BF, tag="wq")
            wk_sb = wp.tile([128, KT, DC], BF, tag="wk")
            wv_sb = wp.tile([128, KT, DC], BF, tag="wv")
            bq_sb = wp.tile([DC, 1], F32, tag="bq")
            bk_sb = wp.tile([DC, 1], F32, tag="bk")
            wo_sb = wp.tile([DC, E], BF, tag="wo")
            # wq + bq first so the first projection piece can start ASAP;
            # xin(0, 0) is issued right after wq (see prologue below)
            nc.sync.dma_start(wq_sb, wqT_d.ap().rearrange("(k p) d -> p k d", p=128))
            nc.sync.dma_start(bq_sb, bq_d.ap())
            preamble_rest = []
            preamble_rest.append(lambda: nc.sync.dma_start(
                wk_sb, wkT_d.ap().rearrange("(k p) d -> p k d", p=128)))
            preamble_rest.append(lambda: nc.sync.dma_start(bk_sb, bk_d.ap()))
            preamble_rest.append(lambda: nc.sync.dma_start(
                wv_sb, wvT_d.ap().rearrange("(k p) d -> p k d", p=128)))
            preamble_rest.append(lambda: nc.sync.dma_start(wo_sb, woT_d.ap()))
            ident = wp.tile([128, 128], BF, tag="ident")
            make_identity(nc, ident)
            if causal:
                tri_sb = wp.tile([128, 128], BF, tag="tri")
                preamble_rest.append(lambda: nc.sync.dma_start(tri_sb, tri_d.ap()))

            # ---- per-batch persistent-ish tiles (rotated via pool bufs) ----
            qT = {}    # bb -> [128 d, T] bf16
            kT = {}
            vn = {}    # bb -> [128 s-part, NST*130] bf16  (v | 1 | v | 1 per s-tile)
            atT = {}   # bb -> [128 d, T] bf16 attention output, transposed
            xin = {}   # (bb, c, t) -> [128, KT, 512] bf16

            def issue_xin(bb, c):
                if bb >= NB or (bb, c, "q") in xin:
                    return
                b = bb % B
                for t in ("q", "k", "v"):
                    xt = mp.tile([128, KT, 512], BF, tag=f"x{t}", bufs=3,
                                 name=f"x{t}_{bb}_{c}")
                    src = src_map[t].ap().rearrange("(k p) r -> p k r", p=128)
                    nc.sync.dma_start(
                        xt, src[:, :, b * T + 512 * c : b * T + 512 * (c + 1)]
                    )
                    xin[(bb, c, t)] = xt

            def proj_qk_piece(bb, c, t):
                # q/k projection for column chunk c -> qT/kT[bb][:, 512c:512c+512]
                def emit():
                    w_sb = wq_sb if t == "q" else wk_sb
                    bias = bq_sb if t == "q" else bk_sb
                    scale = SCALE if t == "q" else 1.0
                    dst = qT[bb] if t == "q" else kT[bb]
                    xt = xin.pop((bb, c, t))
                    pps = ps.tile([128, 512], F32, tag="pp", bufs=1, name=f"pp_{t}{bb}{c}")
                    for k in range(KT):
                        nc.tensor.matmul(
                            pps, w_sb[:, k, :], xt[:, k, :],
                            start=(k == 0), stop=(k == KT - 1),
                        )
                    nc.vector.tensor_scalar(
                        dst[:, 512 * c : 512 * (c + 1)], pps, bias, scale, add, mult
                    )
                return emit

            def proj_v_piece(bb, c, jjs):
                # v projection for s-tiles [4c+jj for jj in jjs] -> vn[bb] natural
                def emit():
                    xt = xin[(bb, c, "v")]
                    pps = ps.tile([128, 512], F32, tag="pp", bufs=1, name=f"pp_v{bb}{c}{jjs[0]}")
                    for jj in jjs:
                        for k in range(KT):
                            nc.tensor.matmul(
                                pps[:, 128 * jj : 128 * (jj + 1)],
                                xt[:, k, 128 * jj : 128 * (jj + 1)],
                                wv_sb[:, k, :],
                                start=(k == 0), stop=(k == KT - 1),
                            )
                    for jj in jjs:
                        j = 4 * c + jj
                        # both heads in one strided op (ones col skipped)
                        dstv = vn[bb][:, 130 * j : 130 * j + 130].rearrange(
                            "p (two c2) -> p two c2", two=2)[:, :, 0:64]
                        srcv = pps[:, 128 * jj : 128 * (jj + 1)].rearrange(
                            "p (two c2) -> p two c2", two=2)
                        nc.vector.tensor_copy(dstv, srcv)
                    if jjs[-1] == 3 and c == NCH - 1:
                        pass
                return emit

            def alloc_batch(bb):
                qT[bb] = mp.tile([DC, T], BF, tag="qTb", name=f"qT{bb}")
                kT[bb] = mp.tile([DC, T], BF, tag="kTb", name=f"kT{bb}")
                vn[bb] = mp.tile([128, NST * 130], BF, tag="vnat", name=f"vn{bb}")
                atT[bb] = mp.tile([DC, T], BF, tag="atT", name=f"atT{bb}")
                vv = vn[bb].rearrange("p (j c) -> p j c", c=65)
                nc.vector.memset(vv[:, :, 64], 1.0)

            def proj_pieces(bb):
                # all projection pieces for batch bb, chunk-major
                out = []
                for c in range(NCH):
                    out.append(proj_qk_piece(bb, c, "q"))
                    out.append(proj_qk_piece(bb, c, "k"))
                    out.append(proj_v_piece(bb, c, (0, 1)))
                    out.append(proj_v_piece(bb, c, (2, 3)))
                return out

            osb_cur = [None]

            def outproj_piece(bb, rr, n):
                # half n of the output projection for global r-tile rr
                def emit():
                    b = bb % B
                    ops_t = ps.tile([128, 512], F32, tag="pt" if n == 0 else "pp",
                                    bufs=1, name=f"ops{bb}_{rr}_{n}")
                    nc.tensor.matmul(
                        ops_t,
                        atT[bb][:, 128 * rr : 128 * (rr + 1)],
                        wo_sb[:, 512 * n : 512 * (n + 1)],
                        start=True, stop=True,
                    )
                    if n == 0:
                        osb_cur[0] = mp.tile([128, E], BF, tag="osb", bufs=4,
                                             name=f"osb{bb}_{rr}")
                    o_sb = osb_cur[0]
                    nc.vector.tensor_copy(o_sb[:, 512 * n : 512 * (n + 1)], ops_t)
                    if n == 1:
                        nc.sync.dma_start(
                            out_d.ap()[b * T + 128 * rr : b * T + 128 * (rr + 1), :],
                            o_sb,
                        )
                return emit

            def attn_chunk(bb, c, pieces):
                # attention for q-chunk c of batch bb; `pieces` are deferred
                # emitters (projections of bb+1, outproj of previous chunk)
                # interleaved into the s-tile loop to keep PE busy during exp.
                n_s = 4 * (c + 1) if causal else NST
                pi = 0
                at_ps = [
                    ps.tile([65, 512], F32, tag=f"at{h}", bufs=1,
                            name=f"at{h}_{bb}_{c}")
                    for h in range(HPC)
                ]
                # spread pieces across the s-tile loop
                for j in range(n_s):
                    diag = causal and j >= 4 * c
                    sc = ps.tile([128, 1024], F32, tag="sc", name=f"sc{bb}_{c}_{j}")
                    for h in range(HPC):
                        nc.tensor.matmul(
                            sc[:, 512 * h : 512 * (h + 1)],
                            kT[bb][64 * h : 64 * h + 64, 128 * j : 128 * (j + 1)],
                            qT[bb][64 * h : 64 * h + 64, 512 * c : 512 * (c + 1)],
                            start=True, stop=True,
                        )
                    if diag:
                        qo = 128 * (j - 4 * c)
                        for h in range(HPC):
                            nc.tensor.matmul(
                                sc[:, 512 * h + qo : 512 * h + qo + 128],
                                ident, tri_sb,
                                start=False, stop=True, skip_group_check=True,
                            )
                    pT = mp.tile([128, 1024], BF, tag="pT", bufs=4,
                                 name=f"pT{bb}_{c}_{j}")
                    nc.scalar.activation(pT, sc, Exp)
                    if not causal:
                        em = mp.tile([128, 512], BF, tag="em", bufs=3,
                                     name=f"em{bb}_{c}_{j}")
                        nc.sync.dma_start(
                            em,
                            em_d.ap()[128 * j : 128 * (j + 1),
                                      512 * c : 512 * (c + 1)],
                        )
                        pm = mp.tile([128, 1024], BF, tag="pm", bufs=4,
                                     name=f"pm{bb}_{c}_{j}")
                        for h in range(HPC):
                            nc.vector.tensor_tensor(
                                pm[:, 512 * h : 512 * (h + 1)],
                                pT[:, 512 * h : 512 * (h + 1)], em, mult,
                            )
                        pT = pm
                    # interleave deferred pieces while ACT runs exp; spread
                    # them so none bunch up at the end of short chunks
                    want = -(-(len(pieces) - pi) // (n_s - j)) if j < n_s else 0
                    for _ in range(max(want, 0)):
                        if pi < len(pieces):
                            pieces[pi]()
                            pi += 1
                    # PV: transposed layout (out [65, 512] per head) — two big
                    # matmuls per s-tile instead of eight tiny ones, avoiding
                    # the per-matmul weight-load tax on HW.  Masked (above-
                    # diagonal) probabilities are exactly 0 after the tri-add,
                    # so accumulating every j is correct; row 64 (ones column
                    # of vn) accumulates the softmax denominator.
                    qo = 128 * (j - 4 * c) if diag else 0
                    for h in range(HPC):
                        nc.tensor.matmul(
                            at_ps[h][:, qo:512],
                            vn[bb][:, 130 * j + 65 * h : 130 * j + 65 * (h + 1)],
                            pT[:, 512 * h + qo : 512 * (h + 1)],
                            start=(j == 0), stop=(j == n_s - 1),
                            skip_group_check=True,
                        )
                # normalization: per head, reciprocal of the denominator row,
                # broadcast down the partitions, multiply into attnT
                for h in range(HPC):
                    rl = mp.tile([1, 512], BF, tag="rl", name=f"rl{bb}_{c}_{h}")
                    with nc.allow_low_precision(reason="softmax denom recip"):
                        nc.vector.reciprocal(rl, at_ps[h][64:65, :])
                    rlb = mp.tile([64, 512], BF, tag="rlb", name=f"rlb{bb}_{c}_{h}")
                    nc.gpsimd.partition_broadcast(rlb, rl)
                    nc.vector.tensor_tensor(
                        atT[bb][64 * h : 64 * h + 64, 512 * c : 512 * (c + 1)],
                        at_ps[h][0:64, :], rlb, mult,
                    )
                # leftover pieces
                while pi < len(pieces):
                    pieces[pi]()
                    pi += 1

            # ---------------- main pipelined schedule ----------------
            # prologue: batch 0 loaded and projected up front; remaining
            # weight DMAs interleaved behind the first xin chunk
            issue_xin(0, 0)
            for fn in preamble_rest:
                fn()
            for c in range(1, NCH):
                issue_xin(0, c)
            issue_xin(1, 0)
            alloc_batch(0)
            for p in proj_pieces(0):
                p()

            pending_outproj = []   # r-tile pieces of the previous chunk
            for bb in range(NB):
                if bb + 1 < NB:
                    alloc_batch(bb + 1)
                    next_proj = proj_pieces(bb + 1)
                else:
                    next_proj = []
                for c in range(NCH):
                    # xin prefetch for the window after next
                    if c < NCH - 1:
                        issue_xin(bb + 1, c + 1)
                    else:
                        issue_xin(bb + 2, 0)
                    # interleave outproj halves with projection pieces of
                    # batch bb+1 so consecutive pieces use different PSUM tags
                    take = list(next_proj[4 * c : 4 * (c + 1)])
                    ops = list(pending_outproj)
                    pieces = []
                    while take or ops:
                        if take:
                            pieces.append(take.pop(0))
                        for _ in range(2):
                            if ops:
                                pieces.append(ops.pop(0))
                    attn_chunk(bb, c, pieces)
                    pending_outproj = [
                        outproj_piece(bb, 4 * c + r, n)
                        for r in range(4) for n in range(2)
                    ]
            for p in pending_outproj:
                p()

    nc.compile()
    return nc


def _causal_mask_ref():
    return np.where(
        np.arange(T)[:, None] >= np.arange(T)[None, :], np.float32(0.0), np.float32(-1e9)
    ).astype(np.float32)


def _tri_pattern():
    # additive causal triangle for a diagonal 128x128 block:
    # tri[s, q] = 0 if s <= q else NEG
    s = np.arange(128)[:, None]
    q = np.arange(128)[None, :]
    return np.where(s <= q, np.float32(0.0), np.float32(NEG)).astype(BF16)


def _prep_in_maps(query, key, value, attn_mask, wq, bq, wk, bk, wv, bv, wo, causal):
    # [T, B, E] -> [E, B*T] b-major columns, bf16
    qT = np.ascontiguousarray(query.transpose(2, 1, 0).reshape(E, R)).astype(BF16)
    kT = np.ascontiguousarray(key.transpose(2, 1, 0).reshape(E, R)).astype(BF16)
    vT = np.ascontiguousarray(value.transpose(2, 1, 0).reshape(E, R)).astype(BF16)
    common = {"qT": qT, "kT": kT, "vT": vT}
    if causal:
        common["tri"] = np.ascontiguousarray(_tri_pattern())
    else:
        common["emaskT"] = np.exp(attn_mask.astype(np.float64).T).astype(BF16)
    in_maps = []
    for c in range(NCORES):
        sl = slice(DC * c, DC * (c + 1))
        m = dict(common)
        m["wqT"] = np.ascontiguousarray(wq[sl, :].T).astype(BF16)
        m["wkT"] = np.ascontiguousarray(wk[sl, :].T).astype(BF16)
        m["wvT"] = np.ascontiguousarray(wv[sl, :].T).astype(BF16)
        m["woT"] = np.ascontiguousarray(wo[:, sl].T).astype(BF16)
        m["bq"] = bq[sl].astype(np.float32)[:, None]
        m["bk"] = bk[sl].astype(np.float32)[:, None]
        in_maps.append(m)
    return in_maps


def _postprocess(results, bo_eff):
    acc = results[0]["out"].astype(np.float32)
    for c in range(1, NCORES):
        acc = acc + results[c]["out"].astype(np.float32)
    out = acc.reshape(B, T, E).transpose(1, 0, 2) + bo_eff[None, None, :]
    return np.ascontiguousarray(out.astype(np.float32))


def kernel(query, key, value, attn_mask, wq, bq, wk, bk, wv, bv, wo, bo):
    assert query.shape == (T, B, E), query.shape
    causal = bool(np.array_equal(attn_mask, _causal_mask_ref()))
    if causal not in _CACHE:
        _CACHE[causal] = _build(causal)
    nc = _CACHE[causal]
    in_maps = _prep_in_maps(
        query, key, value, attn_mask, wq, bq, wk, bk, wv, bv, wo, causal
    )
    res = run_bass_kernel_spmd(nc, in_maps, core_ids=list(range(NCORES)))
    # bv passes through softmax unchanged (rows sum to 1), so its effect on
    # the output is the constant bv @ wo.T — folded into the output bias.
    bo_eff = (
        np.asarray(bo, dtype=np.float64)
        + np.asarray(bv, dtype=np.float64) @ np.asarray(wo, dtype=np.float64).T
    ).astype(np.float32)
    return _postprocess(res.results, bo_eff)
